# revision 29
# baseline (speedup 1.0000x reference)
import sys

if "/opt/trn_rl_repo" not in sys.path:
    sys.path.insert(0, "/opt/trn_rl_repo")

import numpy as np

B, T, C = 2, 2048, 2048
H, H_KV = 16, 8
D = C // H  # 128
NCORES = 8
HL = H // NCORES  # 2 local query heads per core; 1 kv head per core

F32R_SCALE = 0.08838834764831845  # 1/sqrt(128)


def build_nc(b=B, t=T, c=C, mmdt="bf16"):
    """Build the per-core Bass program. Same program on all 8 cores; the
    sharding lives entirely in the input data each core receives."""
    import concourse.bass as bass  # noqa: F401
    import concourse.mybir as mybir
    import concourse.tile as tile
    from concourse import bacc

    f32 = mybir.dt.float32
    f32r = mybir.dt.float32r if mmdt == "f32r" else mybir.dt.bfloat16
    EXP = mybir.ActivationFunctionType.Exp

    ncb = c // 128  # contraction blocks for projections
    nt = t // 512  # 512-wide q tiles
    njb_per_t = 512 // 128  # 4 k-blocks per 512 q-tile

    nc = bacc.Bacc("TRN2", target_bir_lowering=False, debug=False)

    xT = nc.dram_tensor("xT", [b, c, t], f32r, kind="ExternalInput")
    wq = nc.dram_tensor("wq", [c, HL * D], f32r, kind="ExternalInput")
    wk = nc.dram_tensor("wk", [c, D], f32r, kind="ExternalInput")
    wv = nc.dram_tensor("wv", [c, D], f32r, kind="ExternalInput")
    wp = nc.dram_tensor("wp", [HL * D, c], f32r, kind="ExternalInput")
    cos2 = nc.dram_tensor("cos2", [128, t], f32r, kind="ExternalInput")
    sin2 = nc.dram_tensor("sin2", [128, t], f32r, kind="ExternalInput")
    maskf = nc.dram_tensor("maskf", [128, 512], f32r, kind="ExternalInput")
    onesv = nc.dram_tensor("onesv", [128, 4], f32r, kind="ExternalInput")
    ident = nc.dram_tensor("ident", [128, 128], f32, kind="ExternalInput")
    y = nc.dram_tensor("y", [b, t, c], f32r, kind="ExternalOutput")

    with tile.TileContext(nc) as tc:
        with (
            tc.tile_pool(name="wts", bufs=1) as wpool,
            tc.tile_pool(name="data", bufs=1) as dpool,
            tc.tile_pool(name="work", bufs=2) as wkp,
            tc.tile_pool(name="psum", bufs=1, space="PSUM") as pp,
        ):
            # ---- resident weights / tables (scalar DMA queue; x + y use sync).
            # Emission order = queue order: wq chunks first (needed by the
            # first pass), then the first i4-pair's odd x tiles (splitting the
            # head x feed across both DMA queues), then the rest by need-time.
            nw = max(ncb // 4, 1)  # cb chunks per weight DMA
            wq_sbs, wk_sbs, wv_sbs = [], [], []
            for wi in range(ncb // nw):
                cbs = slice(wi * nw * 128, (wi + 1) * nw * 128)
                wq_i = wpool.tile([128, nw * HL * D], f32r, name=f"wq{wi}")
                nc.scalar.dma_start(
                    wq_i[:].rearrange("p (cb d) -> p cb d", d=HL * D),
                    wq[cbs, :].rearrange("(cb p) d -> p cb d", p=128),
                )
                wq_sbs.append(wq_i)
            # Skinny head tiles: batch 0's first two i4 tiles arrive just in
            # time (the head is HBM-bound), split across both DMA queues. Odd
            # cb tiles ride the scalar queue under their own tag: tag rings
            # serialize allocations across queues, so a shared tag would chain
            # the two queues head-to-tail.
            XTS_pre = {}
            for cb in range(ncb):
                if cb % 2:
                    xts0 = wkp.tile([128, 512], f32r, tag="xto", bufs=8, name="xts0")
                    nc.scalar.dma_start(xts0[:], xT[0, cb * 128 : (cb + 1) * 128, 0:512])
                else:
                    xts0 = wkp.tile([128, 512], f32r, tag="xt", bufs=20, name="xts0")
                    nc.sync.dma_start(xts0[:], xT[0, cb * 128 : (cb + 1) * 128, 0:512])
                XTS_pre[(0, cb)] = xts0
            for cb in range(ncb):
                xts1 = wkp.tile([128, 512], f32r, tag="xt", bufs=20, name="xts1")
                nc.sync.dma_start(xts1[:], xT[0, cb * 128 : (cb + 1) * 128, 512:1024])
                XTS_pre[(1, cb)] = xts1
            for wi in range(ncb // nw):
                cbs = slice(wi * nw * 128, (wi + 1) * nw * 128)
                wk_i = wpool.tile([128, nw * D], f32r, name=f"wk{wi}")
                nc.scalar.dma_start(
                    wk_i[:].rearrange("p (cb d) -> p cb d", d=D),
                    wk[cbs, :].rearrange("(cb p) d -> p cb d", p=128),
                )
                wk_sbs.append(wk_i)
            cos_sb = wpool.tile([128, t], f32r)
            nc.scalar.dma_start(cos_sb[:], cos2[:, :])
            sin_sb = wpool.tile([128, t], f32r)
            nc.scalar.dma_start(sin_sb[:], sin2[:, :])
            for wi in range(ncb // nw):
                cbs = slice(wi * nw * 128, (wi + 1) * nw * 128)
                wv_i = wpool.tile([128, nw * D], f32r, name=f"wv{wi}")
                nc.scalar.dma_start(
                    wv_i[:].rearrange("p (cb d) -> p cb d", d=D),
                    wv[cbs, :].rearrange("(cb p) d -> p cb d", p=128),
                )
                wv_sbs.append(wv_i)
            mask_sb = wpool.tile([128, 512], f32r)
            nc.scalar.dma_start(mask_sb[:], maskf[:, :])
            ones_sb = wpool.tile([128, 4], f32r)
            nc.scalar.dma_start(ones_sb[:], onesv[:, :])
            id_sb = wpool.tile([128, 128], f32)
            nc.scalar.dma_start(id_sb[:], ident[:, :])
            wp_sb = wpool.tile([128, HL * c], f32r)  # [p, (f, cout)]
            nc.scalar.dma_start(
                wp_sb[:].rearrange("p (f n) -> p f n", n=c),
                wp.rearrange("(f p) n -> p f n", p=128),
            )
            warm = wpool.tile([128, 1], f32)
            nc.scalar.activation(warm[:], cos_sb[:, 0:1], EXP, scale=1.0)

            swap_mask = [i ^ 1 for i in range(32)]

            def rope(dest, src, ts_):
                # dest = src*cosI + swap_adjacent(src)*sinS (pair-interleaved
                # head layout: host permuted Wq/Wk cols so rotate-half pairs
                # are adjacent partitions)
                ra = wkp.tile([128, 512], f32, tag="ra", bufs=2)
                rb = wkp.tile([128, 512], f32, tag="rb", bufs=2)
                nc.vector.tensor_mul(ra[:], src, cos_sb[:, ts_])
                nc.vector.stream_shuffle(rb[:], src, swap_mask)
                nc.vector.tensor_mul(rb[:], rb[:], sin_sb[:, ts_])
                nc.vector.tensor_add(dest, ra[:], rb[:])

            for bi in range(b):
                # ---- per-batch persistent tiles ----
                QT = [dpool.tile([128, t], f32r, tag=f"qt{h}", name=f"QT{h}") for h in range(HL)]
                KT = dpool.tile([128, t], f32r, tag="kt")
                VT = dpool.tile([128, t], f32, tag="vtt")
                Vn = dpool.tile([128, t], f32r, tag="vn")  # V natural [k, (jb d)]
                AT = [dpool.tile([128, t], f32r, tag=f"at{h}", name=f"AT{h}") for h in range(HL)]

                # ---- x tiles: one DMA per (i4-pair, cb) — fatter transfers
                # amortize the per-partition descriptor cost on the sync queue
                XTP = {}
                for p4 in range(nt // 2):
                    if bi == 0 and p4 == 0:
                        continue  # covered by the skinny head tiles
                    for cb in range(ncb):
                        xtc = wkp.tile([128, 1024], f32r, tag="xt", bufs=20, name="xtc")
                        nc.sync.dma_start(
                            xtc[:],
                            xT[bi, cb * 128 : (cb + 1) * 128, p4 * 1024 : (p4 + 1) * 1024],
                        )
                        XTP[(p4, cb)] = xtc

                def xt_slice(i4, cb, last):
                    if bi == 0 and i4 < 2:
                        t_ = XTS_pre.pop((i4, cb)) if last else XTS_pre[(i4, cb)]
                        return t_[:, :]
                    tile_ = XTP.pop((i4 // 2, cb)) if (last and i4 % 2 == 1) else XTP[(i4 // 2, cb)]
                    lo = (i4 % 2) * 512
                    return tile_[:, lo : lo + 512]

                # ---- QKV projections, kind-major passes (+ fused RoPE) ----
                def emit_transposes(i4):
                    ts_ = slice(i4 * 512, (i4 + 1) * 512)
                    pt = pp.tile([128, 512], f32, tag="pav", bufs=2)
                    for jj in range(4):
                        nc.tensor.transpose(
                            pt[:, jj * 128 : (jj + 1) * 128],
                            VT[:, i4 * 512 + jj * 128 : i4 * 512 + (jj + 1) * 128],
                            id_sb[:],
                        )
                    nc.vector.tensor_copy(Vn[:, ts_], pt[:])

                pend_tp = None
                for i4 in range(nt):
                    ts_ = slice(i4 * 512, (i4 + 1) * 512)
                    # four kind-major passes (q0, q1, k, v), each re-reading the
                    # resident x tiles, so each pass's RoPE/copy overlaps the
                    # next pass's matmuls.
                    def pass_(kind):
                        ps = pp.tile([128, 512], f32, tag="mm", bufs=3, name=f"ps{kind}")
                        for cb in range(ncb):
                            xtr = xt_slice(i4, cb, last=(kind == "v"))
                            st, sp = (cb == 0), (cb == ncb - 1)
                            wi, cbl = cb // nw, cb % nw
                            base = cbl * HL * D
                            w_sb = {
                                "q0": lambda: wq_sbs[wi][:, base : base + 128],
                                "q1": lambda: wq_sbs[wi][:, base + 128 : base + 256],
                                "k": lambda: wk_sbs[wi][:, cbl * 128 : (cbl + 1) * 128],
                                "v": lambda: wv_sbs[wi][:, cbl * 128 : (cbl + 1) * 128],
                            }[kind]()
                            nc.tensor.matmul(ps[:], w_sb, xtr, start=st, stop=sp)
                        return ps

                    pq0 = pass_("q0")
                    if pend_tp is not None:
                        emit_transposes(pend_tp)
                    rope(QT[0][:, ts_], pq0[:], ts_)
                    pq1 = pass_("q1")
                    rope(QT[1][:, ts_], pq1[:], ts_)
                    pk = pass_("k")
                    rope(KT[:, ts_], pk[:], ts_)
                    pv = pass_("v")
                    nc.scalar.copy(VT[:, ts_], pv[:])
                    pend_tp = i4
                emit_transposes(pend_tp)

                # ---- attention (j-pipelined: QK of j runs while exp of j-1 is
                # consumed by den/AV) with interleaved out-projection units ----
                yrows = {}

                def emit_oproj_unit(it, n):
                    # one (row-block, col-slice) of the output projection; the
                    # po matmuls have no exp dependency, so they soak up PE
                    # bubbles in the attention j-loop
                    if n == 0:
                        yrows[it] = wkp.tile(
                            [128, c], f32r, tag="yout", bufs=4, name="yrow"
                        )
                    po = pp.tile([128, 512], f32, tag="po", bufs=2, name="po")
                    for hh in range(HL):
                        nc.tensor.matmul(
                            po[:],
                            AT[hh][:, it * 128 : (it + 1) * 128],
                            wp_sb[:, hh * c + n * 512 : hh * c + (n + 1) * 512],
                            start=(hh == 0), stop=(hh == HL - 1),
                        )
                    dst = yrows[it][:, n * 512 : (n + 1) * 512]
                    nc.vector.tensor_copy(dst, po[:])
                    if n == 3:
                        nc.sync.dma_start(
                            y[bi, it * 128 : (it + 1) * 128, :], yrows.pop(it)[:]
                        )

                def attn(i4, pending):
                    qs = slice(i4 * 512, (i4 + 1) * 512)
                    njb = njb_per_t * (i4 + 1)
                    pav = [pp.tile([128, 512], f32, tag="pav", bufs=2, name=f"pav{h}") for h in range(HL)]
                    pden2 = pp.tile([2, 512], f32, tag="den", bufs=1, name="pden2")
                    E2 = {}
                    offs = {}
                    for j in range(njb + 1):
                        if j < njb:
                            off = max(j - njb_per_t * i4, 0) * 128
                            offs[j] = off
                            for h in range(HL):
                                pst = pp.tile([128, 512], f32, tag="mm", bufs=3, name=f"pst{h}")
                                nc.tensor.matmul(
                                    pst[:, off:512],
                                    KT[:, j * 128 : (j + 1) * 128],
                                    QT[h][:, i4 * 512 + off : (i4 + 1) * 512],
                                    start=True, stop=True,
                                )
                                E_ = wkp.tile([128, 512], f32r, tag="E", bufs=8, name=f"E{h}")
                                nc.scalar.activation(
                                    E_[:, off:512], pst[:, off:512], EXP,
                                    scale=F32R_SCALE,
                                )
                                if j >= njb_per_t * i4:
                                    # zero strictly-lower triangle of diag block
                                    nc.vector.tensor_mul(
                                        E_[:, off : off + 128],
                                        E_[:, off : off + 128],
                                        mask_sb[:, 384:512],
                                    )
                                E2[(j, h)] = E_
                        if j > 0:
                            jp = j - 1
                            o = offs[jp]
                            Eps = [E2.pop((jp, h)) for h in range(HL)]
                            for h in range(HL):
                                nc.tensor.matmul(
                                    pden2[:, o:512],
                                    ones_sb[:, 2 * h : 2 * h + 2],
                                    Eps[h][:, o:512],
                                    start=(jp == 0 and h == 0),
                                    stop=(jp == njb - 1 and h == HL - 1),
                                    skip_group_check=True,
                                )
                            for h in range(HL):
                                nc.tensor.matmul(
                                    pav[h][:, o:512],
                                    Vn[:, jp * 128 : (jp + 1) * 128],
                                    Eps[h][:, o:512],
                                    start=(jp == 0), stop=(jp == njb - 1),
                                    skip_group_check=True,
                                )
                        iters_left = njb + 1 - j
                        k = min(
                            len(pending),
                            max(1, -(-len(pending) // max(iters_left, 1))),
                        )
                        for _ in range(k):
                            emit_oproj_unit(*pending.pop(0))
                    rec2 = wkp.tile([2, 512], f32, tag="rec", bufs=2)
                    nc.vector.reciprocal_approx_fast(rec2[:], pden2[:, :])
                    rec1 = wkp.tile([1, 512], f32, tag="rec1", bufs=2)
                    nc.scalar.dma_start(rec1[:], rec2[1:2, :])
                    for h in range(HL):
                        rbc = wkp.tile([128, 512], f32, tag="rbc", bufs=2)
                        nc.gpsimd.partition_broadcast(
                            rbc[:], rec2[0:1, :] if h == 0 else rec1[:]
                        )
                        nc.vector.tensor_mul(AT[h][:, qs], pav[h][:], rbc[:])

                def oproj_units(i4):
                    return [
                        (it, n)
                        for it in range(i4 * 4, (i4 + 1) * 4)
                        for n in range(4)
                    ]

                for i4 in range(nt):
                    attn(i4, oproj_units(i4 - 1) if i4 > 0 else [])
                for unit in oproj_units(nt - 1):
                    emit_oproj_unit(*unit)

    nc.compile()
    return nc


def host_inputs(x, Wq, Wk, Wv, Wp, ncores=NCORES, mmdt="bf16"):
    import ml_dtypes

    mdt = np.float32 if mmdt == "f32r" else ml_dtypes.bfloat16
    """Per-core input dicts (sharding + layout prep on host)."""
    b, t, c = x.shape
    d = D
    xT = np.ascontiguousarray(np.transpose(x, (0, 2, 1)))  # [B, C, T]
    inv = (1.0 / (10000.0 ** (np.arange(0, d, 2, dtype=np.float32) / np.float32(d)))).astype(np.float32)
    pos = np.arange(t, dtype=np.float32)
    fr = np.outer(pos, inv).astype(np.float32)  # [T, 64]
    cosT = np.cos(fr).T.astype(np.float32)  # [64, T]
    sinT = np.sin(fr).T.astype(np.float32)
    # pair-interleaved rope tables: partition 2m,2m+1 <- freq m; sign -/+ on sin
    cosI = np.ascontiguousarray(np.repeat(cosT, 2, axis=0))  # [128, T]
    sinS = np.ascontiguousarray(
        np.stack([-sinT, sinT], axis=1).reshape(128, t)
    )
    # column permutation putting rope pair (m, m+64) at (2m, 2m+1), per head
    perm = np.stack([np.arange(64), np.arange(64) + 64], 1).reshape(128)
    triu = np.triu(np.ones((128, 128), np.float32))
    maskf = np.ascontiguousarray(
        np.concatenate([np.zeros((128, 384), np.float32), triu], 1)
    )
    onesv = np.concatenate(
        [
            np.ones((128, 1), np.float32),
            np.zeros((128, 2), np.float32),
            np.ones((128, 1), np.float32),
        ],
        axis=1,
    )
    ident = np.eye(128, dtype=np.float32)

    def permute_heads(w):
        # w: [c, nheads*d] -> same with each head's columns permuted by perm
        nh = w.shape[1] // d
        wv_ = w.reshape(w.shape[0], nh, d)
        return np.ascontiguousarray(wv_[:, :, perm].reshape(w.shape))

    Wq_p = permute_heads(Wq)
    Wk_p = permute_heads(Wk)

    xTm = xT.astype(mdt) if mdt is not np.float32 else xT
    in_maps = []
    for ci in range(ncores):
        qs = slice(ci * HL * d, (ci + 1) * HL * d)
        in_maps.append(
            {
                "xT": xTm,
                "wq": np.ascontiguousarray(Wq_p[:, qs]).astype(mdt),
                "wk": np.ascontiguousarray(Wk_p[:, ci * d : (ci + 1) * d]).astype(mdt),
                "wv": np.ascontiguousarray(Wv[:, ci * d : (ci + 1) * d]).astype(mdt),
                "wp": np.ascontiguousarray(Wp[qs, :]).astype(mdt),
                "cos2": cosI.astype(mdt),
                "sin2": sinS.astype(mdt),
                "maskf": maskf.astype(mdt),
                "onesv": onesv.astype(mdt),
                "ident": ident,
            }
        )
    return in_maps


_NC_CACHE = {}

MMDT = "bf16"


def _get_nc(mmdt=None):
    mmdt = mmdt or MMDT
    key = (B, T, C, mmdt)
    if key not in _NC_CACHE:
        _NC_CACHE[key] = build_nc(B, T, C, mmdt=mmdt)
    return _NC_CACHE[key]


def _install_cc_error_surfacing():
    """Make neuronx_cc hook failures print a real traceback instead of the
    opaque PJRT 'py_result' error."""
    try:
        from concourse import bass2jax

        bass2jax.install_neuronx_cc_hook()
        import libneuronxla

        if getattr(libneuronxla, "_tb_wrapped", False):
            return
        inner = libneuronxla.neuronx_cc

        def wrapped(*a, **k):
            try:
                return inner(*a, **k)
            except BaseException:
                import traceback

                traceback.print_exc()
                raise

        libneuronxla.neuronx_cc = wrapped
        libneuronxla._tb_wrapped = True
    except Exception:
        pass


def run_spmd(x, Wq, Wk, Wv, Wp, trace=False, mmdt=None):
    from concourse.bass_utils import run_bass_kernel_spmd

    mmdt = mmdt or MMDT
    _install_cc_error_surfacing()

    nc = _get_nc(mmdt)
    in_maps = host_inputs(x, Wq, Wk, Wv, Wp, mmdt=mmdt)
    last_err = None
    for attempt in range(3):
        try:
            res = run_bass_kernel_spmd(
                nc, in_maps, core_ids=list(range(NCORES)), trace=trace
            )
            break
        except Exception as e:  # transient NRT device faults: retry
            last_err = e
            import time as _time

            _time.sleep(5.0)
    else:
        raise last_err
    acc = res.results[0]["y"].astype(np.float64)
    for i in range(1, NCORES):
        acc += res.results[i]["y"]
    return acc.astype(np.float32), res


def kernel(x, Wq, Wk, Wv, Wp):
    out, _ = run_spmd(x, Wq, Wk, Wv, Wp, trace=False)
    return out


# revision 30
# speedup vs baseline: 1.1435x; 1.1435x over previous
import sys

if "/opt/trn_rl_repo" not in sys.path:
    sys.path.insert(0, "/opt/trn_rl_repo")

import numpy as np

B, T, C = 2, 2048, 2048
H, H_KV = 16, 8
D = C // H  # 128
NCORES = 8
HL = H // NCORES  # 2 local query heads per core; 1 kv head per core

F32R_SCALE = 0.08838834764831845  # 1/sqrt(128)


def build_nc(b=B, t=T, c=C, mmdt="bf16"):
    """Build the per-core Bass program. Same program on all 8 cores; the
    sharding lives entirely in the input data each core receives."""
    import concourse.bass as bass  # noqa: F401
    import concourse.mybir as mybir
    import concourse.tile as tile
    from concourse import bacc

    f32 = mybir.dt.float32
    f32r = mybir.dt.float32r if mmdt == "f32r" else mybir.dt.bfloat16
    EXP = mybir.ActivationFunctionType.Exp

    ncb = c // 128  # contraction blocks for projections
    nt = t // 512  # 512-wide q tiles
    njb_per_t = 512 // 128  # 4 k-blocks per 512 q-tile

    nc = bacc.Bacc("TRN2", target_bir_lowering=False, debug=False)

    xT = nc.dram_tensor("xT", [b, c, t], f32r, kind="ExternalInput")
    wq = nc.dram_tensor("wq", [c, HL * D], f32r, kind="ExternalInput")
    wk = nc.dram_tensor("wk", [c, D], f32r, kind="ExternalInput")
    wv = nc.dram_tensor("wv", [c, D], f32r, kind="ExternalInput")
    wp = nc.dram_tensor("wp", [HL * D, c], f32r, kind="ExternalInput")
    cos2 = nc.dram_tensor("cos2", [128, t], f32r, kind="ExternalInput")
    sin2 = nc.dram_tensor("sin2", [128, t], f32r, kind="ExternalInput")
    maskf = nc.dram_tensor("maskf", [128, 512], f32r, kind="ExternalInput")
    onesv = nc.dram_tensor("onesv", [128, 4], f32r, kind="ExternalInput")
    ident = nc.dram_tensor("ident", [128, 128], f32, kind="ExternalInput")
    y = nc.dram_tensor("y", [b, t, c], f32r, kind="ExternalOutput")

    with tile.TileContext(nc) as tc:
        with (
            tc.tile_pool(name="wts", bufs=1) as wpool,
            tc.tile_pool(name="data", bufs=1) as dpool,
            tc.tile_pool(name="work", bufs=2) as wkp,
            tc.tile_pool(name="psum", bufs=1, space="PSUM") as pp,
        ):
            # ---- resident weights / tables (scalar DMA queue; x + y use sync).
            # Emission order = queue order: wq chunks first (needed by the
            # first pass), then the first i4-pair's odd x tiles (splitting the
            # head x feed across both DMA queues), then the rest by need-time.
            nw = max(ncb // 4, 1)  # cb chunks per weight DMA
            wq_sbs, wk_sbs, wv_sbs = [], [], []
            for wi in range(ncb // nw):
                cbs = slice(wi * nw * 128, (wi + 1) * nw * 128)
                wq_i = wpool.tile([128, nw * HL * D], f32r, name=f"wq{wi}")
                nc.scalar.dma_start(
                    wq_i[:].rearrange("p (cb d) -> p cb d", d=HL * D),
                    wq[cbs, :].rearrange("(cb p) d -> p cb d", p=128),
                )
                wq_sbs.append(wq_i)
            # Head x tiles for batch 0's first i4-pair, split across both DMA
            # queues (odd cb on scalar under a separate tag: tag rings
            # serialize allocations across queues, so a shared tag would chain
            # the two queues head-to-tail).
            XTP_pre = {}
            for cb in range(ncb):
                if cb % 2:
                    xtc0 = wkp.tile([128, 1024], f32r, tag="xto", bufs=8, name="xtc0")
                    nc.scalar.dma_start(xtc0[:], xT[0, cb * 128 : (cb + 1) * 128, 0:1024])
                else:
                    xtc0 = wkp.tile([128, 1024], f32r, tag="xt", bufs=20, name="xtc0")
                    nc.sync.dma_start(xtc0[:], xT[0, cb * 128 : (cb + 1) * 128, 0:1024])
                XTP_pre[(0, cb)] = xtc0
            for wi in range(ncb // nw):
                cbs = slice(wi * nw * 128, (wi + 1) * nw * 128)
                wk_i = wpool.tile([128, nw * D], f32r, name=f"wk{wi}")
                nc.scalar.dma_start(
                    wk_i[:].rearrange("p (cb d) -> p cb d", d=D),
                    wk[cbs, :].rearrange("(cb p) d -> p cb d", p=128),
                )
                wk_sbs.append(wk_i)
            cos_sb = wpool.tile([128, t], f32r)
            nc.scalar.dma_start(cos_sb[:], cos2[:, :])
            sin_sb = wpool.tile([128, t], f32r)
            nc.scalar.dma_start(sin_sb[:], sin2[:, :])
            for wi in range(ncb // nw):
                cbs = slice(wi * nw * 128, (wi + 1) * nw * 128)
                wv_i = wpool.tile([128, nw * D], f32r, name=f"wv{wi}")
                nc.scalar.dma_start(
                    wv_i[:].rearrange("p (cb d) -> p cb d", d=D),
                    wv[cbs, :].rearrange("(cb p) d -> p cb d", p=128),
                )
                wv_sbs.append(wv_i)
            mask_sb = wpool.tile([128, 512], f32r)
            nc.scalar.dma_start(mask_sb[:], maskf[:, :])
            ones_sb = wpool.tile([128, 4], f32r)
            nc.scalar.dma_start(ones_sb[:], onesv[:, :])
            id_sb = wpool.tile([128, 128], f32)
            nc.scalar.dma_start(id_sb[:], ident[:, :])
            wp_sb = wpool.tile([128, HL * c], f32r)  # [p, (f, cout)]
            nc.scalar.dma_start(
                wp_sb[:].rearrange("p (f n) -> p f n", n=c),
                wp.rearrange("(f p) n -> p f n", p=128),
            )
            warm = wpool.tile([128, 1], f32)
            nc.scalar.activation(warm[:], cos_sb[:, 0:1], EXP, scale=1.0)

            swap_mask = [i ^ 1 for i in range(32)]

            def rope(dest, src, ts_):
                # dest = src*cosI + swap_adjacent(src)*sinS (pair-interleaved
                # head layout: host permuted Wq/Wk cols so rotate-half pairs
                # are adjacent partitions)
                ra = wkp.tile([128, 512], f32, tag="ra", bufs=2)
                rb = wkp.tile([128, 512], f32, tag="rb", bufs=2)
                nc.vector.tensor_mul(ra[:], src, cos_sb[:, ts_])
                nc.vector.stream_shuffle(rb[:], src, swap_mask)
                nc.vector.tensor_mul(rb[:], rb[:], sin_sb[:, ts_])
                nc.vector.tensor_add(dest, ra[:], rb[:])

            for bi in range(b):
                # ---- per-batch persistent tiles ----
                QT = [dpool.tile([128, t], f32r, tag=f"qt{h}", name=f"QT{h}") for h in range(HL)]
                KT = dpool.tile([128, t], f32r, tag="kt")
                VT = dpool.tile([128, t], f32, tag="vtt")
                Vn = dpool.tile([128, t], f32r, tag="vn")  # V natural [k, (jb d)]
                AT = [dpool.tile([128, t], f32r, tag=f"at{h}", name=f"AT{h}") for h in range(HL)]

                # ---- x tiles: one DMA per (i4-pair, cb) — fatter transfers
                # amortize the per-partition descriptor cost on the sync queue
                XTP = {}
                for p4 in range(nt // 2):
                    if bi == 0 and p4 == 0:
                        for cb in range(ncb):
                            XTP[(p4, cb)] = XTP_pre[(0, cb)]
                        continue
                    for cb in range(ncb):
                        xtc = wkp.tile([128, 1024], f32r, tag="xt", bufs=20, name="xtc")
                        nc.sync.dma_start(
                            xtc[:],
                            xT[bi, cb * 128 : (cb + 1) * 128, p4 * 1024 : (p4 + 1) * 1024],
                        )
                        XTP[(p4, cb)] = xtc

                def xt_slice(i4, cb, last):
                    tile_ = XTP.pop((i4 // 2, cb)) if (last and i4 % 2 == 1) else XTP[(i4 // 2, cb)]
                    lo = (i4 % 2) * 512
                    return tile_[:, lo : lo + 512]

                # ---- QKV projections, kind-major passes (+ fused RoPE) ----
                def emit_transposes(i4):
                    ts_ = slice(i4 * 512, (i4 + 1) * 512)
                    pt = pp.tile([128, 512], f32, tag="pav", bufs=2)
                    for jj in range(4):
                        nc.tensor.transpose(
                            pt[:, jj * 128 : (jj + 1) * 128],
                            VT[:, i4 * 512 + jj * 128 : i4 * 512 + (jj + 1) * 128],
                            id_sb[:],
                        )
                    nc.vector.tensor_copy(Vn[:, ts_], pt[:])

                pend_tp = None
                for i4 in range(nt):
                    ts_ = slice(i4 * 512, (i4 + 1) * 512)
                    # four kind-major passes (q0, q1, k, v), each re-reading the
                    # resident x tiles, so each pass's RoPE/copy overlaps the
                    # next pass's matmuls.
                    def pass_(kind):
                        ps = pp.tile([128, 512], f32, tag="mm", bufs=3, name=f"ps{kind}")
                        for cb in range(ncb):
                            xtr = xt_slice(i4, cb, last=(kind == "v"))
                            st, sp = (cb == 0), (cb == ncb - 1)
                            wi, cbl = cb // nw, cb % nw
                            base = cbl * HL * D
                            w_sb = {
                                "q0": lambda: wq_sbs[wi][:, base : base + 128],
                                "q1": lambda: wq_sbs[wi][:, base + 128 : base + 256],
                                "k": lambda: wk_sbs[wi][:, cbl * 128 : (cbl + 1) * 128],
                                "v": lambda: wv_sbs[wi][:, cbl * 128 : (cbl + 1) * 128],
                            }[kind]()
                            nc.tensor.matmul(ps[:], w_sb, xtr, start=st, stop=sp)
                        return ps

                    pq0 = pass_("q0")
                    if pend_tp is not None:
                        emit_transposes(pend_tp)
                    rope(QT[0][:, ts_], pq0[:], ts_)
                    pq1 = pass_("q1")
                    rope(QT[1][:, ts_], pq1[:], ts_)
                    pk = pass_("k")
                    rope(KT[:, ts_], pk[:], ts_)
                    pv = pass_("v")
                    nc.scalar.copy(VT[:, ts_], pv[:])
                    pend_tp = i4
                emit_transposes(pend_tp)

                # ---- attention (j-pipelined: QK of j runs while exp of j-1 is
                # consumed by den/AV) with interleaved out-projection units ----
                yrows = {}

                def emit_oproj_unit(it, n):
                    # one (row-block, col-slice) of the output projection; the
                    # po matmuls have no exp dependency, so they soak up PE
                    # bubbles in the attention j-loop
                    if n == 0:
                        yrows[it] = wkp.tile(
                            [128, c], f32r, tag="yout", bufs=4, name="yrow"
                        )
                    po = pp.tile([128, 512], f32, tag="po", bufs=2, name="po")
                    for hh in range(HL):
                        nc.tensor.matmul(
                            po[:],
                            AT[hh][:, it * 128 : (it + 1) * 128],
                            wp_sb[:, hh * c + n * 512 : hh * c + (n + 1) * 512],
                            start=(hh == 0), stop=(hh == HL - 1),
                        )
                    dst = yrows[it][:, n * 512 : (n + 1) * 512]
                    nc.vector.tensor_copy(dst, po[:])
                    if n == 3:
                        nc.sync.dma_start(
                            y[bi, it * 128 : (it + 1) * 128, :], yrows.pop(it)[:]
                        )

                def attn(i4, pending):
                    qs = slice(i4 * 512, (i4 + 1) * 512)
                    njb = njb_per_t * (i4 + 1)
                    pav = [pp.tile([128, 512], f32, tag="pav", bufs=2, name=f"pav{h}") for h in range(HL)]
                    pden2 = pp.tile([2, 512], f32, tag="den", bufs=1, name="pden2")
                    E2 = {}
                    offs = {}
                    for j in range(njb + 1):
                        if j < njb:
                            off = max(j - njb_per_t * i4, 0) * 128
                            offs[j] = off
                            for h in range(HL):
                                pst = pp.tile([128, 512], f32, tag="mm", bufs=3, name=f"pst{h}")
                                nc.tensor.matmul(
                                    pst[:, off:512],
                                    KT[:, j * 128 : (j + 1) * 128],
                                    QT[h][:, i4 * 512 + off : (i4 + 1) * 512],
                                    start=True, stop=True,
                                )
                                E_ = wkp.tile([128, 512], f32r, tag="E", bufs=8, name=f"E{h}")
                                nc.scalar.activation(
                                    E_[:, off:512], pst[:, off:512], EXP,
                                    scale=F32R_SCALE,
                                )
                                if j >= njb_per_t * i4:
                                    # zero strictly-lower triangle of diag block
                                    nc.vector.tensor_mul(
                                        E_[:, off : off + 128],
                                        E_[:, off : off + 128],
                                        mask_sb[:, 384:512],
                                    )
                                E2[(j, h)] = E_
                        if j > 0:
                            jp = j - 1
                            o = offs[jp]
                            Eps = [E2.pop((jp, h)) for h in range(HL)]
                            for h in range(HL):
                                nc.tensor.matmul(
                                    pden2[:, o:512],
                                    ones_sb[:, 2 * h : 2 * h + 2],
                                    Eps[h][:, o:512],
                                    start=(jp == 0 and h == 0),
                                    stop=(jp == njb - 1 and h == HL - 1),
                                    skip_group_check=True,
                                )
                            for h in range(HL):
                                nc.tensor.matmul(
                                    pav[h][:, o:512],
                                    Vn[:, jp * 128 : (jp + 1) * 128],
                                    Eps[h][:, o:512],
                                    start=(jp == 0), stop=(jp == njb - 1),
                                    skip_group_check=True,
                                )
                        iters_left = njb + 1 - j
                        k = min(
                            len(pending),
                            max(1, -(-len(pending) // max(iters_left, 1))),
                        )
                        for _ in range(k):
                            emit_oproj_unit(*pending.pop(0))
                    rec2 = wkp.tile([2, 512], f32, tag="rec", bufs=2)
                    nc.vector.reciprocal_approx_fast(rec2[:], pden2[:, :])
                    rec1 = wkp.tile([1, 512], f32, tag="rec1", bufs=2)
                    nc.scalar.dma_start(rec1[:], rec2[1:2, :])
                    for h in range(HL):
                        rbc = wkp.tile([128, 512], f32, tag="rbc", bufs=2)
                        nc.gpsimd.partition_broadcast(
                            rbc[:], rec2[0:1, :] if h == 0 else rec1[:]
                        )
                        nc.vector.tensor_mul(AT[h][:, qs], pav[h][:], rbc[:])

                def oproj_units(i4):
                    return [
                        (it, n)
                        for it in range(i4 * 4, (i4 + 1) * 4)
                        for n in range(4)
                    ]

                for i4 in range(nt):
                    attn(i4, oproj_units(i4 - 1) if i4 > 0 else [])
                for unit in oproj_units(nt - 1):
                    emit_oproj_unit(*unit)

    nc.compile()
    return nc


def host_inputs(x, Wq, Wk, Wv, Wp, ncores=NCORES, mmdt="bf16"):
    import ml_dtypes

    mdt = np.float32 if mmdt == "f32r" else ml_dtypes.bfloat16
    """Per-core input dicts (sharding + layout prep on host)."""
    b, t, c = x.shape
    d = D
    xT = np.ascontiguousarray(np.transpose(x, (0, 2, 1)))  # [B, C, T]
    inv = (1.0 / (10000.0 ** (np.arange(0, d, 2, dtype=np.float32) / np.float32(d)))).astype(np.float32)
    pos = np.arange(t, dtype=np.float32)
    fr = np.outer(pos, inv).astype(np.float32)  # [T, 64]
    cosT = np.cos(fr).T.astype(np.float32)  # [64, T]
    sinT = np.sin(fr).T.astype(np.float32)
    # pair-interleaved rope tables: partition 2m,2m+1 <- freq m; sign -/+ on sin
    cosI = np.ascontiguousarray(np.repeat(cosT, 2, axis=0))  # [128, T]
    sinS = np.ascontiguousarray(
        np.stack([-sinT, sinT], axis=1).reshape(128, t)
    )
    # column permutation putting rope pair (m, m+64) at (2m, 2m+1), per head
    perm = np.stack([np.arange(64), np.arange(64) + 64], 1).reshape(128)
    triu = np.triu(np.ones((128, 128), np.float32))
    maskf = np.ascontiguousarray(
        np.concatenate([np.zeros((128, 384), np.float32), triu], 1)
    )
    onesv = np.concatenate(
        [
            np.ones((128, 1), np.float32),
            np.zeros((128, 2), np.float32),
            np.ones((128, 1), np.float32),
        ],
        axis=1,
    )
    ident = np.eye(128, dtype=np.float32)

    def permute_heads(w):
        # w: [c, nheads*d] -> same with each head's columns permuted by perm
        nh = w.shape[1] // d
        wv_ = w.reshape(w.shape[0], nh, d)
        return np.ascontiguousarray(wv_[:, :, perm].reshape(w.shape))

    Wq_p = permute_heads(Wq)
    Wk_p = permute_heads(Wk)

    xTm = xT.astype(mdt) if mdt is not np.float32 else xT
    in_maps = []
    for ci in range(ncores):
        qs = slice(ci * HL * d, (ci + 1) * HL * d)
        in_maps.append(
            {
                "xT": xTm,
                "wq": np.ascontiguousarray(Wq_p[:, qs]).astype(mdt),
                "wk": np.ascontiguousarray(Wk_p[:, ci * d : (ci + 1) * d]).astype(mdt),
                "wv": np.ascontiguousarray(Wv[:, ci * d : (ci + 1) * d]).astype(mdt),
                "wp": np.ascontiguousarray(Wp[qs, :]).astype(mdt),
                "cos2": cosI.astype(mdt),
                "sin2": sinS.astype(mdt),
                "maskf": maskf.astype(mdt),
                "onesv": onesv.astype(mdt),
                "ident": ident,
            }
        )
    return in_maps


_NC_CACHE = {}

MMDT = "bf16"


def _get_nc(mmdt=None):
    mmdt = mmdt or MMDT
    key = (B, T, C, mmdt)
    if key not in _NC_CACHE:
        _NC_CACHE[key] = build_nc(B, T, C, mmdt=mmdt)
    return _NC_CACHE[key]


def _install_cc_error_surfacing():
    """Make neuronx_cc hook failures print a real traceback instead of the
    opaque PJRT 'py_result' error."""
    try:
        from concourse import bass2jax

        bass2jax.install_neuronx_cc_hook()
        import libneuronxla

        if getattr(libneuronxla, "_tb_wrapped", False):
            return
        inner = libneuronxla.neuronx_cc

        def wrapped(*a, **k):
            try:
                return inner(*a, **k)
            except BaseException:
                import traceback

                traceback.print_exc()
                raise

        libneuronxla.neuronx_cc = wrapped
        libneuronxla._tb_wrapped = True
    except Exception:
        pass


def run_spmd(x, Wq, Wk, Wv, Wp, trace=False, mmdt=None):
    from concourse.bass_utils import run_bass_kernel_spmd

    mmdt = mmdt or MMDT
    _install_cc_error_surfacing()

    nc = _get_nc(mmdt)
    in_maps = host_inputs(x, Wq, Wk, Wv, Wp, mmdt=mmdt)
    last_err = None
    for attempt in range(3):
        try:
            res = run_bass_kernel_spmd(
                nc, in_maps, core_ids=list(range(NCORES)), trace=trace
            )
            break
        except Exception as e:  # transient NRT device faults: retry
            last_err = e
            import time as _time

            _time.sleep(5.0)
    else:
        raise last_err
    acc = res.results[0]["y"].astype(np.float64)
    for i in range(1, NCORES):
        acc += res.results[i]["y"]
    return acc.astype(np.float32), res


def kernel(x, Wq, Wk, Wv, Wp):
    out, _ = run_spmd(x, Wq, Wk, Wv, Wp, trace=False)
    return out


# revision 31
# speedup vs baseline: 1.1438x; 1.0003x over previous
import sys

if "/opt/trn_rl_repo" not in sys.path:
    sys.path.insert(0, "/opt/trn_rl_repo")

import numpy as np

B, T, C = 2, 2048, 2048
H, H_KV = 16, 8
D = C // H  # 128
NCORES = 8
HL = H // NCORES  # 2 local query heads per core; 1 kv head per core

F32R_SCALE = 0.08838834764831845  # 1/sqrt(128)


def build_nc(b=B, t=T, c=C, mmdt="bf16"):
    """Build the per-core Bass program. Same program on all 8 cores; the
    sharding lives entirely in the input data each core receives."""
    import concourse.bass as bass  # noqa: F401
    import concourse.mybir as mybir
    import concourse.tile as tile
    from concourse import bacc

    f32 = mybir.dt.float32
    f32r = mybir.dt.float32r if mmdt == "f32r" else mybir.dt.bfloat16
    EXP = mybir.ActivationFunctionType.Exp

    ncb = c // 128  # contraction blocks for projections
    nt = t // 512  # 512-wide q tiles
    njb_per_t = 512 // 128  # 4 k-blocks per 512 q-tile

    nc = bacc.Bacc("TRN2", target_bir_lowering=False, debug=False)

    xT = nc.dram_tensor("xT", [b, c, t], f32r, kind="ExternalInput")
    wq = nc.dram_tensor("wq", [c, HL * D], f32r, kind="ExternalInput")
    wk = nc.dram_tensor("wk", [c, D], f32r, kind="ExternalInput")
    wv = nc.dram_tensor("wv", [c, D], f32r, kind="ExternalInput")
    wp = nc.dram_tensor("wp", [HL * D, c], f32r, kind="ExternalInput")
    cos2 = nc.dram_tensor("cos2", [128, t], f32r, kind="ExternalInput")
    sin2 = nc.dram_tensor("sin2", [128, t], f32r, kind="ExternalInput")
    maskf = nc.dram_tensor("maskf", [128, 512], f32r, kind="ExternalInput")
    onesv = nc.dram_tensor("onesv", [128, 4], f32r, kind="ExternalInput")
    ident = nc.dram_tensor("ident", [128, 128], f32, kind="ExternalInput")
    y = nc.dram_tensor("y", [b, t, c], f32r, kind="ExternalOutput")

    with tile.TileContext(nc) as tc:
        with (
            tc.tile_pool(name="wts", bufs=1) as wpool,
            tc.tile_pool(name="data", bufs=1) as dpool,
            tc.tile_pool(name="work", bufs=2) as wkp,
            tc.tile_pool(name="psum", bufs=1, space="PSUM") as pp,
        ):
            # ---- resident weights / tables (scalar DMA queue; x + y use sync).
            # Emission order = queue order: wq chunks first (needed by the
            # first pass), then the first i4-pair's odd x tiles (splitting the
            # head x feed across both DMA queues), then the rest by need-time.
            nw = max(ncb // 4, 1)  # cb chunks per weight DMA
            wq_sbs, wk_sbs, wv_sbs = [], [], []
            for wi in range(ncb // nw):
                cbs = slice(wi * nw * 128, (wi + 1) * nw * 128)
                wq_i = wpool.tile([128, nw * HL * D], f32r, name=f"wq{wi}")
                nc.scalar.dma_start(
                    wq_i[:].rearrange("p (cb d) -> p cb d", d=HL * D),
                    wq[cbs, :].rearrange("(cb p) d -> p cb d", p=128),
                )
                wq_sbs.append(wq_i)
            # Head x tiles for batch 0's first i4-pair, split across both DMA
            # queues (odd cb on scalar under a separate tag: tag rings
            # serialize allocations across queues, so a shared tag would chain
            # the two queues head-to-tail).
            XTP_pre = {}
            for cb in range(ncb):
                if cb % 2:
                    xtc0 = wkp.tile([128, 1024], f32r, tag="xto", bufs=8, name="xtc0")
                    nc.scalar.dma_start(xtc0[:], xT[0, cb * 128 : (cb + 1) * 128, 0:1024])
                else:
                    xtc0 = wkp.tile([128, 1024], f32r, tag="xt", bufs=20, name="xtc0")
                    nc.sync.dma_start(xtc0[:], xT[0, cb * 128 : (cb + 1) * 128, 0:1024])
                XTP_pre[(0, cb)] = xtc0
            for wi in range(ncb // nw):
                cbs = slice(wi * nw * 128, (wi + 1) * nw * 128)
                wk_i = wpool.tile([128, nw * D], f32r, name=f"wk{wi}")
                nc.scalar.dma_start(
                    wk_i[:].rearrange("p (cb d) -> p cb d", d=D),
                    wk[cbs, :].rearrange("(cb p) d -> p cb d", p=128),
                )
                wk_sbs.append(wk_i)
            cos_sb = wpool.tile([128, t], f32r)
            nc.scalar.dma_start(cos_sb[:], cos2[:, :])
            sin_sb = wpool.tile([128, t], f32r)
            nc.scalar.dma_start(sin_sb[:], sin2[:, :])
            for wi in range(ncb // nw):
                cbs = slice(wi * nw * 128, (wi + 1) * nw * 128)
                wv_i = wpool.tile([128, nw * D], f32r, name=f"wv{wi}")
                nc.scalar.dma_start(
                    wv_i[:].rearrange("p (cb d) -> p cb d", d=D),
                    wv[cbs, :].rearrange("(cb p) d -> p cb d", p=128),
                )
                wv_sbs.append(wv_i)
            mask_sb = wpool.tile([128, 512], f32r)
            nc.scalar.dma_start(mask_sb[:], maskf[:, :])
            ones_sb = wpool.tile([128, 4], f32r)
            nc.scalar.dma_start(ones_sb[:], onesv[:, :])
            id_sb = wpool.tile([128, 128], f32)
            nc.scalar.dma_start(id_sb[:], ident[:, :])
            wp_sb = wpool.tile([128, HL * c], f32r)  # [p, (f, cout)]
            nc.scalar.dma_start(
                wp_sb[:].rearrange("p (f n) -> p f n", n=c),
                wp.rearrange("(f p) n -> p f n", p=128),
            )
            warm = wpool.tile([128, 1], f32)
            nc.scalar.activation(warm[:], cos_sb[:, 0:1], EXP, scale=1.0)

            swap_mask = [i ^ 1 for i in range(32)]

            def rope(dest, src, ts_):
                # dest = src*cosI + swap_adjacent(src)*sinS (pair-interleaved
                # head layout: host permuted Wq/Wk cols so rotate-half pairs
                # are adjacent partitions)
                ra = wkp.tile([128, 512], f32, tag="ra", bufs=2)
                rb = wkp.tile([128, 512], f32, tag="rb", bufs=2)
                nc.vector.tensor_mul(ra[:], src, cos_sb[:, ts_])
                nc.vector.stream_shuffle(rb[:], src, swap_mask)
                nc.vector.tensor_mul(rb[:], rb[:], sin_sb[:, ts_])
                nc.vector.tensor_add(dest, ra[:], rb[:])

            pending_x = []  # cross-batch deferred out-projection units
            yrows = {}
            for bi in range(b):
                # ---- per-batch persistent tiles ----
                QT = [dpool.tile([128, t], f32r, tag=f"qt{h}", name=f"QT{h}") for h in range(HL)]
                KT = dpool.tile([128, t], f32r, tag="kt")
                VT = dpool.tile([128, t], f32, tag="vtt")
                Vn = dpool.tile([128, t], f32r, tag="vn")  # V natural [k, (jb d)]
                AT = [dpool.tile([128, t], f32r, tag=f"at{h}", name=f"AT{h}") for h in range(HL)]

                # ---- x tiles: one DMA per (i4-pair, cb) — fatter transfers
                # amortize the per-partition descriptor cost on the sync queue
                XTP = {}
                for p4 in range(nt // 2):
                    if bi == 0 and p4 == 0:
                        for cb in range(ncb):
                            XTP[(p4, cb)] = XTP_pre[(0, cb)]
                        continue
                    for cb in range(ncb):
                        xtc = wkp.tile([128, 1024], f32r, tag="xt", bufs=20, name="xtc")
                        nc.sync.dma_start(
                            xtc[:],
                            xT[bi, cb * 128 : (cb + 1) * 128, p4 * 1024 : (p4 + 1) * 1024],
                        )
                        XTP[(p4, cb)] = xtc

                def xt_slice(i4, cb, last):
                    tile_ = XTP.pop((i4 // 2, cb)) if (last and i4 % 2 == 1) else XTP[(i4 // 2, cb)]
                    lo = (i4 % 2) * 512
                    return tile_[:, lo : lo + 512]

                # ---- QKV projections, kind-major passes (+ fused RoPE) ----
                def emit_transposes(i4):
                    ts_ = slice(i4 * 512, (i4 + 1) * 512)
                    pt = pp.tile([128, 512], f32, tag="pav", bufs=2)
                    for jj in range(4):
                        nc.tensor.transpose(
                            pt[:, jj * 128 : (jj + 1) * 128],
                            VT[:, i4 * 512 + jj * 128 : i4 * 512 + (jj + 1) * 128],
                            id_sb[:],
                        )
                    nc.vector.tensor_copy(Vn[:, ts_], pt[:])

                pend_tp = None
                for i4 in range(nt):
                    ts_ = slice(i4 * 512, (i4 + 1) * 512)
                    # four kind-major passes (q0, q1, k, v), each re-reading the
                    # resident x tiles, so each pass's RoPE/copy overlaps the
                    # next pass's matmuls.
                    def pass_(kind):
                        ps = pp.tile([128, 512], f32, tag="mm", bufs=3, name=f"ps{kind}")
                        for cb in range(ncb):
                            if cb % 4 == 3 and pending_x:
                                fn = pending_x.pop(0)
                                fn()
                            xtr = xt_slice(i4, cb, last=(kind == "v"))
                            st, sp = (cb == 0), (cb == ncb - 1)
                            wi, cbl = cb // nw, cb % nw
                            base = cbl * HL * D
                            w_sb = {
                                "q0": lambda: wq_sbs[wi][:, base : base + 128],
                                "q1": lambda: wq_sbs[wi][:, base + 128 : base + 256],
                                "k": lambda: wk_sbs[wi][:, cbl * 128 : (cbl + 1) * 128],
                                "v": lambda: wv_sbs[wi][:, cbl * 128 : (cbl + 1) * 128],
                            }[kind]()
                            nc.tensor.matmul(ps[:], w_sb, xtr, start=st, stop=sp)
                        return ps

                    pq0 = pass_("q0")
                    if pend_tp is not None:
                        emit_transposes(pend_tp)
                    rope(QT[0][:, ts_], pq0[:], ts_)
                    pq1 = pass_("q1")
                    rope(QT[1][:, ts_], pq1[:], ts_)
                    pk = pass_("k")
                    rope(KT[:, ts_], pk[:], ts_)
                    pv = pass_("v")
                    nc.scalar.copy(VT[:, ts_], pv[:])
                    pend_tp = i4
                emit_transposes(pend_tp)

                # ---- attention (j-pipelined: QK of j runs while exp of j-1 is
                # consumed by den/AV) with interleaved out-projection units ----
                def emit_oproj_unit(it, n, ATl=None, bil=None, tail=False):
                    # one (row-block, col-slice) of the output projection; the
                    # po matmuls have no exp dependency, so they soak up PE
                    # bubbles in the attention j-loop
                    ATl = AT if ATl is None else ATl
                    bil = bi if bil is None else bil
                    if n == 0:
                        yrows[(bil, it)] = wkp.tile(
                            [128, c], f32r, tag="yout", bufs=4, name="yrow"
                        )
                    po = pp.tile([128, 512], f32, tag="po", bufs=2, name="po")
                    for hh in range(HL):
                        nc.tensor.matmul(
                            po[:],
                            ATl[hh][:, it * 128 : (it + 1) * 128],
                            wp_sb[:, hh * c + n * 512 : hh * c + (n + 1) * 512],
                            start=(hh == 0), stop=(hh == HL - 1),
                        )
                    dst = yrows[(bil, it)][:, n * 512 : (n + 1) * 512]
                    if tail and n % 2 == 0:
                        nc.scalar.copy(dst, po[:])
                    else:
                        nc.vector.tensor_copy(dst, po[:])
                    if n == 3:
                        nc.sync.dma_start(
                            y[bil, it * 128 : (it + 1) * 128, :],
                            yrows.pop((bil, it))[:],
                        )

                def attn(i4, pending):
                    qs = slice(i4 * 512, (i4 + 1) * 512)
                    njb = njb_per_t * (i4 + 1)
                    pav = [pp.tile([128, 512], f32, tag="pav", bufs=2, name=f"pav{h}") for h in range(HL)]
                    pden2 = pp.tile([2, 512], f32, tag="den", bufs=1, name="pden2")
                    E2 = {}
                    offs = {}
                    for j in range(njb + 1):
                        if j < njb:
                            off = max(j - njb_per_t * i4, 0) * 128
                            offs[j] = off
                            for h in range(HL):
                                pst = pp.tile([128, 512], f32, tag="mm", bufs=3, name=f"pst{h}")
                                nc.tensor.matmul(
                                    pst[:, off:512],
                                    KT[:, j * 128 : (j + 1) * 128],
                                    QT[h][:, i4 * 512 + off : (i4 + 1) * 512],
                                    start=True, stop=True,
                                )
                                E_ = wkp.tile([128, 512], f32r, tag="E", bufs=8, name=f"E{h}")
                                nc.scalar.activation(
                                    E_[:, off:512], pst[:, off:512], EXP,
                                    scale=F32R_SCALE,
                                )
                                if j >= njb_per_t * i4:
                                    # zero strictly-lower triangle of diag block
                                    nc.vector.tensor_mul(
                                        E_[:, off : off + 128],
                                        E_[:, off : off + 128],
                                        mask_sb[:, 384:512],
                                    )
                                E2[(j, h)] = E_
                        if j > 0:
                            jp = j - 1
                            o = offs[jp]
                            Eps = [E2.pop((jp, h)) for h in range(HL)]
                            for h in range(HL):
                                nc.tensor.matmul(
                                    pden2[:, o:512],
                                    ones_sb[:, 2 * h : 2 * h + 2],
                                    Eps[h][:, o:512],
                                    start=(jp == 0 and h == 0),
                                    stop=(jp == njb - 1 and h == HL - 1),
                                    skip_group_check=True,
                                )
                            iters_left = njb + 1 - j
                            k = min(
                                len(pending),
                                max(1, -(-len(pending) // max(iters_left, 1))),
                            )
                            for _ in range(k):
                                emit_oproj_unit(*pending.pop(0))
                            for h in range(HL):
                                nc.tensor.matmul(
                                    pav[h][:, o:512],
                                    Vn[:, jp * 128 : (jp + 1) * 128],
                                    Eps[h][:, o:512],
                                    start=(jp == 0), stop=(jp == njb - 1),
                                    skip_group_check=True,
                                )
                    rec2 = wkp.tile([2, 512], f32, tag="rec", bufs=2)
                    nc.vector.reciprocal_approx_fast(rec2[:], pden2[:, :])
                    rec1 = wkp.tile([1, 512], f32, tag="rec1", bufs=2)
                    nc.scalar.dma_start(rec1[:], rec2[1:2, :])
                    for h in range(HL):
                        rbc = wkp.tile([128, 512], f32, tag="rbc", bufs=2)
                        nc.gpsimd.partition_broadcast(
                            rbc[:], rec2[0:1, :] if h == 0 else rec1[:]
                        )
                        nc.vector.tensor_mul(AT[h][:, qs], pav[h][:], rbc[:])

                def oproj_units(i4):
                    return [
                        (it, n)
                        for it in range(i4 * 4, (i4 + 1) * 4)
                        for n in range(4)
                    ]

                for i4 in range(nt):
                    attn(i4, oproj_units(i4 - 1) if i4 > 0 else [])
                if bi < b - 1:
                    for it_, n_ in oproj_units(nt - 1):
                        pending_x.append(
                            lambda it=it_, n=n_, ATl=AT, bil=bi: emit_oproj_unit(
                                it, n, ATl=ATl, bil=bil
                            )
                        )
                else:
                    for unit in oproj_units(nt - 1):
                        emit_oproj_unit(*unit, tail=True)

    nc.compile()
    return nc


def host_inputs(x, Wq, Wk, Wv, Wp, ncores=NCORES, mmdt="bf16"):
    import ml_dtypes

    mdt = np.float32 if mmdt == "f32r" else ml_dtypes.bfloat16
    """Per-core input dicts (sharding + layout prep on host)."""
    b, t, c = x.shape
    d = D
    xT = np.ascontiguousarray(np.transpose(x, (0, 2, 1)))  # [B, C, T]
    inv = (1.0 / (10000.0 ** (np.arange(0, d, 2, dtype=np.float32) / np.float32(d)))).astype(np.float32)
    pos = np.arange(t, dtype=np.float32)
    fr = np.outer(pos, inv).astype(np.float32)  # [T, 64]
    cosT = np.cos(fr).T.astype(np.float32)  # [64, T]
    sinT = np.sin(fr).T.astype(np.float32)
    # pair-interleaved rope tables: partition 2m,2m+1 <- freq m; sign -/+ on sin
    cosI = np.ascontiguousarray(np.repeat(cosT, 2, axis=0))  # [128, T]
    sinS = np.ascontiguousarray(
        np.stack([-sinT, sinT], axis=1).reshape(128, t)
    )
    # column permutation putting rope pair (m, m+64) at (2m, 2m+1), per head
    perm = np.stack([np.arange(64), np.arange(64) + 64], 1).reshape(128)
    triu = np.triu(np.ones((128, 128), np.float32))
    maskf = np.ascontiguousarray(
        np.concatenate([np.zeros((128, 384), np.float32), triu], 1)
    )
    onesv = np.concatenate(
        [
            np.ones((128, 1), np.float32),
            np.zeros((128, 2), np.float32),
            np.ones((128, 1), np.float32),
        ],
        axis=1,
    )
    ident = np.eye(128, dtype=np.float32)

    def permute_heads(w):
        # w: [c, nheads*d] -> same with each head's columns permuted by perm
        nh = w.shape[1] // d
        wv_ = w.reshape(w.shape[0], nh, d)
        return np.ascontiguousarray(wv_[:, :, perm].reshape(w.shape))

    Wq_p = permute_heads(Wq)
    Wk_p = permute_heads(Wk)

    xTm = xT.astype(mdt) if mdt is not np.float32 else xT
    in_maps = []
    for ci in range(ncores):
        qs = slice(ci * HL * d, (ci + 1) * HL * d)
        in_maps.append(
            {
                "xT": xTm,
                "wq": np.ascontiguousarray(Wq_p[:, qs]).astype(mdt),
                "wk": np.ascontiguousarray(Wk_p[:, ci * d : (ci + 1) * d]).astype(mdt),
                "wv": np.ascontiguousarray(Wv[:, ci * d : (ci + 1) * d]).astype(mdt),
                "wp": np.ascontiguousarray(Wp[qs, :]).astype(mdt),
                "cos2": cosI.astype(mdt),
                "sin2": sinS.astype(mdt),
                "maskf": maskf.astype(mdt),
                "onesv": onesv.astype(mdt),
                "ident": ident,
            }
        )
    return in_maps


_NC_CACHE = {}

MMDT = "bf16"


def _get_nc(mmdt=None):
    mmdt = mmdt or MMDT
    key = (B, T, C, mmdt)
    if key not in _NC_CACHE:
        _NC_CACHE[key] = build_nc(B, T, C, mmdt=mmdt)
    return _NC_CACHE[key]


def _install_cc_error_surfacing():
    """Make neuronx_cc hook failures print a real traceback instead of the
    opaque PJRT 'py_result' error."""
    try:
        from concourse import bass2jax

        bass2jax.install_neuronx_cc_hook()
        import libneuronxla

        if getattr(libneuronxla, "_tb_wrapped", False):
            return
        inner = libneuronxla.neuronx_cc

        def wrapped(*a, **k):
            try:
                return inner(*a, **k)
            except BaseException:
                import traceback

                traceback.print_exc()
                raise

        libneuronxla.neuronx_cc = wrapped
        libneuronxla._tb_wrapped = True
    except Exception:
        pass


def run_spmd(x, Wq, Wk, Wv, Wp, trace=False, mmdt=None):
    from concourse.bass_utils import run_bass_kernel_spmd

    mmdt = mmdt or MMDT
    _install_cc_error_surfacing()

    nc = _get_nc(mmdt)
    in_maps = host_inputs(x, Wq, Wk, Wv, Wp, mmdt=mmdt)
    last_err = None
    for attempt in range(3):
        try:
            res = run_bass_kernel_spmd(
                nc, in_maps, core_ids=list(range(NCORES)), trace=trace
            )
            break
        except Exception as e:  # transient NRT device faults: retry
            last_err = e
            import time as _time

            _time.sleep(5.0)
    else:
        raise last_err
    acc = res.results[0]["y"].astype(np.float64)
    for i in range(1, NCORES):
        acc += res.results[i]["y"]
    return acc.astype(np.float32), res


def kernel(x, Wq, Wk, Wv, Wp):
    out, _ = run_spmd(x, Wq, Wk, Wv, Wp, trace=False)
    return out


# revision 32
# speedup vs baseline: 1.1512x; 1.0065x over previous
import sys

if "/opt/trn_rl_repo" not in sys.path:
    sys.path.insert(0, "/opt/trn_rl_repo")

import numpy as np

B, T, C = 2, 2048, 2048
H, H_KV = 16, 8
D = C // H  # 128
NCORES = 8
HL = H // NCORES  # 2 local query heads per core; 1 kv head per core

F32R_SCALE = 0.08838834764831845  # 1/sqrt(128)


def build_nc(b=B, t=T, c=C, mmdt="bf16"):
    """Build the per-core Bass program. Same program on all 8 cores; the
    sharding lives entirely in the input data each core receives."""
    import concourse.bass as bass  # noqa: F401
    import concourse.mybir as mybir
    import concourse.tile as tile
    from concourse import bacc

    f32 = mybir.dt.float32
    f32r = mybir.dt.float32r if mmdt == "f32r" else mybir.dt.bfloat16
    EXP = mybir.ActivationFunctionType.Exp

    ncb = c // 128  # contraction blocks for projections
    nt = t // 512  # 512-wide q tiles
    njb_per_t = 512 // 128  # 4 k-blocks per 512 q-tile

    nc = bacc.Bacc("TRN2", target_bir_lowering=False, debug=False)

    xT = nc.dram_tensor("xT", [b, c, t], f32r, kind="ExternalInput")
    wq = nc.dram_tensor("wq", [c, HL * D], f32r, kind="ExternalInput")
    wk = nc.dram_tensor("wk", [c, D], f32r, kind="ExternalInput")
    wv = nc.dram_tensor("wv", [c, D], f32r, kind="ExternalInput")
    wp = nc.dram_tensor("wp", [HL * D, c], f32r, kind="ExternalInput")
    cos2 = nc.dram_tensor("cos2", [128, t], f32r, kind="ExternalInput")
    sin2 = nc.dram_tensor("sin2", [128, t], f32r, kind="ExternalInput")
    maskf = nc.dram_tensor("maskf", [128, 512], f32r, kind="ExternalInput")
    onesv = nc.dram_tensor("onesv", [128, 4], f32r, kind="ExternalInput")
    ident = nc.dram_tensor("ident", [128, 128], f32, kind="ExternalInput")
    y = nc.dram_tensor("y", [b, t, c], f32r, kind="ExternalOutput")

    with tile.TileContext(nc) as tc:
        with (
            tc.tile_pool(name="wts", bufs=1) as wpool,
            tc.tile_pool(name="data", bufs=1) as dpool,
            tc.tile_pool(name="work", bufs=2) as wkp,
            tc.tile_pool(name="psum", bufs=1, space="PSUM") as pp,
        ):
            # ---- resident weights / tables (scalar DMA queue; x + y use sync).
            # Emission order = queue order: wq chunks first (needed by the
            # first pass), then the first i4-pair's odd x tiles (splitting the
            # head x feed across both DMA queues), then the rest by need-time.
            nw = max(ncb // 4, 1)  # cb chunks per weight DMA
            wq_sbs, wk_sbs, wv_sbs = [], [], []
            for wi in range(ncb // nw):
                cbs = slice(wi * nw * 128, (wi + 1) * nw * 128)
                wq_i = wpool.tile([128, nw * HL * D], f32r, name=f"wq{wi}")
                nc.scalar.dma_start(
                    wq_i[:].rearrange("p (cb d) -> p cb d", d=HL * D),
                    wq[cbs, :].rearrange("(cb p) d -> p cb d", p=128),
                )
                wq_sbs.append(wq_i)
            # Skinny head tiles: batch 0's first two i4s arrive just in time
            # (the head is HBM-bound), split across both DMA queues. Odd cb on
            # scalar under a separate tag: tag rings serialize allocations
            # across queues, so a shared tag would chain them head-to-tail.
            XTS_pre = {}
            for cb in range(ncb):
                if cb % 2:
                    xts0 = wkp.tile([128, 512], f32r, tag="xto", bufs=8, name="xts0")
                    nc.scalar.dma_start(xts0[:], xT[0, cb * 128 : (cb + 1) * 128, 0:512])
                else:
                    xts0 = wkp.tile([128, 512], f32r, tag="xt", bufs=20, name="xts0")
                    nc.sync.dma_start(xts0[:], xT[0, cb * 128 : (cb + 1) * 128, 0:512])
                XTS_pre[(0, cb)] = xts0
            for cb in range(ncb):
                xts1 = wkp.tile([128, 512], f32r, tag="xt", bufs=20, name="xts1")
                nc.sync.dma_start(xts1[:], xT[0, cb * 128 : (cb + 1) * 128, 512:1024])
                XTS_pre[(1, cb)] = xts1
            for wi in range(ncb // nw):
                cbs = slice(wi * nw * 128, (wi + 1) * nw * 128)
                wk_i = wpool.tile([128, nw * D], f32r, name=f"wk{wi}")
                nc.scalar.dma_start(
                    wk_i[:].rearrange("p (cb d) -> p cb d", d=D),
                    wk[cbs, :].rearrange("(cb p) d -> p cb d", p=128),
                )
                wk_sbs.append(wk_i)
            cos_sb = wpool.tile([128, t], f32r)
            nc.scalar.dma_start(cos_sb[:], cos2[:, :])
            sin_sb = wpool.tile([128, t], f32r)
            nc.scalar.dma_start(sin_sb[:], sin2[:, :])
            for wi in range(ncb // nw):
                cbs = slice(wi * nw * 128, (wi + 1) * nw * 128)
                wv_i = wpool.tile([128, nw * D], f32r, name=f"wv{wi}")
                nc.scalar.dma_start(
                    wv_i[:].rearrange("p (cb d) -> p cb d", d=D),
                    wv[cbs, :].rearrange("(cb p) d -> p cb d", p=128),
                )
                wv_sbs.append(wv_i)
            mask_sb = wpool.tile([128, 512], f32r)
            nc.scalar.dma_start(mask_sb[:], maskf[:, :])
            ones_sb = wpool.tile([128, 4], f32r)
            nc.scalar.dma_start(ones_sb[:], onesv[:, :])
            id_sb = wpool.tile([128, 128], f32)
            nc.scalar.dma_start(id_sb[:], ident[:, :])
            wp_sb = wpool.tile([128, HL * c], f32r)  # [p, (f, cout)]
            nc.scalar.dma_start(
                wp_sb[:].rearrange("p (f n) -> p f n", n=c),
                wp.rearrange("(f p) n -> p f n", p=128),
            )
            warm = wpool.tile([128, 1], f32)
            nc.scalar.activation(warm[:], cos_sb[:, 0:1], EXP, scale=1.0)

            swap_mask = [i ^ 1 for i in range(32)]

            def rope(dest, src, ts_):
                # dest = src*cosI + swap_adjacent(src)*sinS (pair-interleaved
                # head layout: host permuted Wq/Wk cols so rotate-half pairs
                # are adjacent partitions)
                ra = wkp.tile([128, 512], f32, tag="ra", bufs=2)
                rb = wkp.tile([128, 512], f32, tag="rb", bufs=2)
                nc.vector.tensor_mul(ra[:], src, cos_sb[:, ts_])
                nc.vector.stream_shuffle(rb[:], src, swap_mask)
                nc.vector.tensor_mul(rb[:], rb[:], sin_sb[:, ts_])
                nc.vector.tensor_add(dest, ra[:], rb[:])

            pending_x = []  # cross-batch deferred out-projection units
            yrows = {}
            for bi in range(b):
                # ---- per-batch persistent tiles ----
                QT = [dpool.tile([128, t], f32r, tag=f"qt{h}", name=f"QT{h}") for h in range(HL)]
                KT = dpool.tile([128, t], f32r, tag="kt")
                VT = dpool.tile([128, t], f32, tag="vtt")
                Vn = dpool.tile([128, t], f32r, tag="vn")  # V natural [k, (jb d)]
                AT = [dpool.tile([128, t], f32r, tag=f"at{h}", name=f"AT{h}") for h in range(HL)]

                # ---- x tiles: one DMA per (i4-pair, cb) — fatter transfers
                # amortize the per-partition descriptor cost on the sync queue
                XTP = {}
                for p4 in range(nt // 2):
                    if bi == 0 and p4 == 0:
                        continue  # covered by the skinny head tiles
                    for cb in range(ncb):
                        xtc = wkp.tile([128, 1024], f32r, tag="xt", bufs=20, name="xtc")
                        nc.sync.dma_start(
                            xtc[:],
                            xT[bi, cb * 128 : (cb + 1) * 128, p4 * 1024 : (p4 + 1) * 1024],
                        )
                        XTP[(p4, cb)] = xtc

                def xt_slice(i4, cb, last):
                    if bi == 0 and i4 < 2:
                        t_ = XTS_pre.pop((i4, cb)) if last else XTS_pre[(i4, cb)]
                        return t_[:, :]
                    tile_ = XTP.pop((i4 // 2, cb)) if (last and i4 % 2 == 1) else XTP[(i4 // 2, cb)]
                    lo = (i4 % 2) * 512
                    return tile_[:, lo : lo + 512]

                # ---- QKV projections, kind-major passes (+ fused RoPE) ----
                def emit_transposes(i4):
                    ts_ = slice(i4 * 512, (i4 + 1) * 512)
                    pt = pp.tile([128, 512], f32, tag="pav", bufs=2)
                    for jj in range(4):
                        nc.tensor.transpose(
                            pt[:, jj * 128 : (jj + 1) * 128],
                            VT[:, i4 * 512 + jj * 128 : i4 * 512 + (jj + 1) * 128],
                            id_sb[:],
                        )
                    nc.vector.tensor_copy(Vn[:, ts_], pt[:])

                pend_tp = None
                for i4 in range(nt):
                    ts_ = slice(i4 * 512, (i4 + 1) * 512)
                    # four kind-major passes (q0, q1, k, v), each re-reading the
                    # resident x tiles, so each pass's RoPE/copy overlaps the
                    # next pass's matmuls.
                    def pass_(kind):
                        ps = pp.tile([128, 512], f32, tag="mm", bufs=3, name=f"ps{kind}")
                        for cb in range(ncb):
                            if cb % 4 == 3 and pending_x:
                                fn = pending_x.pop(0)
                                fn()
                            xtr = xt_slice(i4, cb, last=(kind == "v"))
                            st, sp = (cb == 0), (cb == ncb - 1)
                            wi, cbl = cb // nw, cb % nw
                            base = cbl * HL * D
                            w_sb = {
                                "q0": lambda: wq_sbs[wi][:, base : base + 128],
                                "q1": lambda: wq_sbs[wi][:, base + 128 : base + 256],
                                "k": lambda: wk_sbs[wi][:, cbl * 128 : (cbl + 1) * 128],
                                "v": lambda: wv_sbs[wi][:, cbl * 128 : (cbl + 1) * 128],
                            }[kind]()
                            nc.tensor.matmul(ps[:], w_sb, xtr, start=st, stop=sp)
                        return ps

                    pq0 = pass_("q0")
                    if pend_tp is not None:
                        emit_transposes(pend_tp)
                    rope(QT[0][:, ts_], pq0[:], ts_)
                    pq1 = pass_("q1")
                    rope(QT[1][:, ts_], pq1[:], ts_)
                    pk = pass_("k")
                    rope(KT[:, ts_], pk[:], ts_)
                    pv = pass_("v")
                    nc.scalar.copy(VT[:, ts_], pv[:])
                    pend_tp = i4
                emit_transposes(pend_tp)

                # ---- attention (j-pipelined: QK of j runs while exp of j-1 is
                # consumed by den/AV) with interleaved out-projection units ----
                def emit_oproj_unit(it, n, ATl=None, bil=None, tail=False):
                    # one (row-block, col-slice) of the output projection; the
                    # po matmuls have no exp dependency, so they soak up PE
                    # bubbles in the attention j-loop
                    ATl = AT if ATl is None else ATl
                    bil = bi if bil is None else bil
                    if n == 0:
                        yrows[(bil, it)] = wkp.tile(
                            [128, c], f32r, tag="yout", bufs=4, name="yrow"
                        )
                    po = pp.tile([128, 512], f32, tag="po", bufs=2, name="po")
                    for hh in range(HL):
                        nc.tensor.matmul(
                            po[:],
                            ATl[hh][:, it * 128 : (it + 1) * 128],
                            wp_sb[:, hh * c + n * 512 : hh * c + (n + 1) * 512],
                            start=(hh == 0), stop=(hh == HL - 1),
                        )
                    dst = yrows[(bil, it)][:, n * 512 : (n + 1) * 512]
                    if tail and n % 2 == 0:
                        nc.scalar.copy(dst, po[:])
                    else:
                        nc.vector.tensor_copy(dst, po[:])
                    if n == 3:
                        nc.sync.dma_start(
                            y[bil, it * 128 : (it + 1) * 128, :],
                            yrows.pop((bil, it))[:],
                        )

                def attn(i4, pending):
                    qs = slice(i4 * 512, (i4 + 1) * 512)
                    njb = njb_per_t * (i4 + 1)
                    pav = [pp.tile([128, 512], f32, tag="pav", bufs=2, name=f"pav{h}") for h in range(HL)]
                    pden2 = pp.tile([2, 512], f32, tag="den", bufs=1, name="pden2")
                    E2 = {}
                    offs = {}
                    for j in range(njb + 1):
                        if j < njb:
                            off = max(j - njb_per_t * i4, 0) * 128
                            offs[j] = off
                            for h in range(HL):
                                pst = pp.tile([128, 512], f32, tag="mm", bufs=3, name=f"pst{h}")
                                nc.tensor.matmul(
                                    pst[:, off:512],
                                    KT[:, j * 128 : (j + 1) * 128],
                                    QT[h][:, i4 * 512 + off : (i4 + 1) * 512],
                                    start=True, stop=True,
                                )
                                E_ = wkp.tile([128, 512], f32r, tag="E", bufs=8, name=f"E{h}")
                                nc.scalar.activation(
                                    E_[:, off:512], pst[:, off:512], EXP,
                                    scale=F32R_SCALE,
                                )
                                if j >= njb_per_t * i4:
                                    # zero strictly-lower triangle of diag block
                                    nc.vector.tensor_mul(
                                        E_[:, off : off + 128],
                                        E_[:, off : off + 128],
                                        mask_sb[:, 384:512],
                                    )
                                E2[(j, h)] = E_
                        if j > 0:
                            jp = j - 1
                            o = offs[jp]
                            Eps = [E2.pop((jp, h)) for h in range(HL)]
                            for h in range(HL):
                                nc.tensor.matmul(
                                    pden2[:, o:512],
                                    ones_sb[:, 2 * h : 2 * h + 2],
                                    Eps[h][:, o:512],
                                    start=(jp == 0 and h == 0),
                                    stop=(jp == njb - 1 and h == HL - 1),
                                    skip_group_check=True,
                                )
                            iters_left = njb + 1 - j
                            k = min(
                                len(pending),
                                max(1, -(-len(pending) // max(iters_left, 1))),
                            )
                            for _ in range(k):
                                emit_oproj_unit(*pending.pop(0))
                            for h in range(HL):
                                nc.tensor.matmul(
                                    pav[h][:, o:512],
                                    Vn[:, jp * 128 : (jp + 1) * 128],
                                    Eps[h][:, o:512],
                                    start=(jp == 0), stop=(jp == njb - 1),
                                    skip_group_check=True,
                                )
                    rec2 = wkp.tile([2, 512], f32, tag="rec", bufs=2)
                    nc.vector.reciprocal_approx_fast(rec2[:], pden2[:, :])
                    rec1 = wkp.tile([1, 512], f32, tag="rec1", bufs=2)
                    nc.scalar.dma_start(rec1[:], rec2[1:2, :])
                    for h in range(HL):
                        rbc = wkp.tile([128, 512], f32, tag="rbc", bufs=2)
                        nc.gpsimd.partition_broadcast(
                            rbc[:], rec2[0:1, :] if h == 0 else rec1[:]
                        )
                        nc.vector.tensor_mul(AT[h][:, qs], pav[h][:], rbc[:])

                def oproj_units(i4):
                    return [
                        (it, n)
                        for it in range(i4 * 4, (i4 + 1) * 4)
                        for n in range(4)
                    ]

                for i4 in range(nt):
                    attn(i4, oproj_units(i4 - 1) if i4 > 0 else [])
                if bi < b - 1:
                    for it_, n_ in oproj_units(nt - 1):
                        pending_x.append(
                            lambda it=it_, n=n_, ATl=AT, bil=bi: emit_oproj_unit(
                                it, n, ATl=ATl, bil=bil
                            )
                        )
                else:
                    for unit in oproj_units(nt - 1):
                        emit_oproj_unit(*unit, tail=True)

    nc.compile()
    return nc


def host_inputs(x, Wq, Wk, Wv, Wp, ncores=NCORES, mmdt="bf16"):
    import ml_dtypes

    mdt = np.float32 if mmdt == "f32r" else ml_dtypes.bfloat16
    """Per-core input dicts (sharding + layout prep on host)."""
    b, t, c = x.shape
    d = D
    xT = np.ascontiguousarray(np.transpose(x, (0, 2, 1)))  # [B, C, T]
    inv = (1.0 / (10000.0 ** (np.arange(0, d, 2, dtype=np.float32) / np.float32(d)))).astype(np.float32)
    pos = np.arange(t, dtype=np.float32)
    fr = np.outer(pos, inv).astype(np.float32)  # [T, 64]
    cosT = np.cos(fr).T.astype(np.float32)  # [64, T]
    sinT = np.sin(fr).T.astype(np.float32)
    # pair-interleaved rope tables: partition 2m,2m+1 <- freq m; sign -/+ on sin
    cosI = np.ascontiguousarray(np.repeat(cosT, 2, axis=0))  # [128, T]
    sinS = np.ascontiguousarray(
        np.stack([-sinT, sinT], axis=1).reshape(128, t)
    )
    # column permutation putting rope pair (m, m+64) at (2m, 2m+1), per head
    perm = np.stack([np.arange(64), np.arange(64) + 64], 1).reshape(128)
    triu = np.triu(np.ones((128, 128), np.float32))
    maskf = np.ascontiguousarray(
        np.concatenate([np.zeros((128, 384), np.float32), triu], 1)
    )
    onesv = np.concatenate(
        [
            np.ones((128, 1), np.float32),
            np.zeros((128, 2), np.float32),
            np.ones((128, 1), np.float32),
        ],
        axis=1,
    )
    ident = np.eye(128, dtype=np.float32)

    def permute_heads(w):
        # w: [c, nheads*d] -> same with each head's columns permuted by perm
        nh = w.shape[1] // d
        wv_ = w.reshape(w.shape[0], nh, d)
        return np.ascontiguousarray(wv_[:, :, perm].reshape(w.shape))

    Wq_p = permute_heads(Wq)
    Wk_p = permute_heads(Wk)

    xTm = xT.astype(mdt) if mdt is not np.float32 else xT
    in_maps = []
    for ci in range(ncores):
        qs = slice(ci * HL * d, (ci + 1) * HL * d)
        in_maps.append(
            {
                "xT": xTm,
                "wq": np.ascontiguousarray(Wq_p[:, qs]).astype(mdt),
                "wk": np.ascontiguousarray(Wk_p[:, ci * d : (ci + 1) * d]).astype(mdt),
                "wv": np.ascontiguousarray(Wv[:, ci * d : (ci + 1) * d]).astype(mdt),
                "wp": np.ascontiguousarray(Wp[qs, :]).astype(mdt),
                "cos2": cosI.astype(mdt),
                "sin2": sinS.astype(mdt),
                "maskf": maskf.astype(mdt),
                "onesv": onesv.astype(mdt),
                "ident": ident,
            }
        )
    return in_maps


_NC_CACHE = {}

MMDT = "bf16"


def _get_nc(mmdt=None):
    mmdt = mmdt or MMDT
    key = (B, T, C, mmdt)
    if key not in _NC_CACHE:
        _NC_CACHE[key] = build_nc(B, T, C, mmdt=mmdt)
    return _NC_CACHE[key]


def _install_cc_error_surfacing():
    """Make neuronx_cc hook failures print a real traceback instead of the
    opaque PJRT 'py_result' error."""
    try:
        from concourse import bass2jax

        bass2jax.install_neuronx_cc_hook()
        import libneuronxla

        if getattr(libneuronxla, "_tb_wrapped", False):
            return
        inner = libneuronxla.neuronx_cc

        def wrapped(*a, **k):
            try:
                return inner(*a, **k)
            except BaseException:
                import traceback

                traceback.print_exc()
                raise

        libneuronxla.neuronx_cc = wrapped
        libneuronxla._tb_wrapped = True
    except Exception:
        pass


def run_spmd(x, Wq, Wk, Wv, Wp, trace=False, mmdt=None):
    from concourse.bass_utils import run_bass_kernel_spmd

    mmdt = mmdt or MMDT
    _install_cc_error_surfacing()

    nc = _get_nc(mmdt)
    in_maps = host_inputs(x, Wq, Wk, Wv, Wp, mmdt=mmdt)
    last_err = None
    for attempt in range(3):
        try:
            res = run_bass_kernel_spmd(
                nc, in_maps, core_ids=list(range(NCORES)), trace=trace
            )
            break
        except Exception as e:  # transient NRT device faults: retry
            last_err = e
            import time as _time

            _time.sleep(5.0)
    else:
        raise last_err
    acc = res.results[0]["y"].astype(np.float64)
    for i in range(1, NCORES):
        acc += res.results[i]["y"]
    return acc.astype(np.float32), res


def kernel(x, Wq, Wk, Wv, Wp):
    out, _ = run_spmd(x, Wq, Wk, Wv, Wp, trace=False)
    return out
